# revision 2
# baseline (speedup 1.0000x reference)
"""Cross-attention kernel for Trainium2, 8-core SPMD.

v4: fp8 hi/lo DoubleRow projections + early exp stream.

Problem (all fp32):
  x [2, 2048, 1024]; wq/wk/wv/w_proj [1024, 1024]; b_proj [1024]
  q = x[:, :1024] @ wq.T   (16 heads x 64)
  k, v = x @ wk.T, x @ wv.T
  out = softmax(q k^T / 8) v  -> proj + bias  -> [2, 1024, 1024]

Sharding: 8 cores = 2 (batch) x 4 (head-groups of 4 heads = 2 pairs of 2).
Each core emits TWO bf16 partials (one per head-pair); host upcasts, sums
the 16 partials per batch and adds the bias.

Design notes (cost model: matmul = out-cols x 0.4167ns x cpr; bf16 cpr=1;
fp8e4 DoubleRow cpr=0.5 with 256-deep contraction -> 4x less PE time per
contraction than bf16; ACT activation = free-cols x 0.833ns, dtype-blind):
  - The ACT exp stream (64 x [128,1024] tiles, ~1.07us each) is the pacing
    engine: ~68us.  Everything else hides under it.
  - q/k/v projections are 3-pass fp8 hi/lo DoubleRow: x = xh+xl, w64 = 64*w
    = wh+wl (the 64x pre-scale keeps w out of fp8's subnormal range;
    1/64^2 folds into the exp scale, 1/64 for v folds into w_proj which is
    an exact bf16 exponent shift).  out = xh@wh + xl@wh + xh@wl; the
    dropped xl@wl term is ~0.1% -- overall rel err ~2.9e-3, better than
    the all-bf16 baseline (4.8e-3).  PE cost 0.75x of bf16.
  - x ships pre-split/pre-swizzled from host as 4 fp8 tensors (xa/xb =
    token halves, hi/lo) in partition-major layout so each chunk-pair is
    ONE 2KB-row DMA (HWDGE gen is exclusive+625ns per DMA instr: fewer,
    bigger DMAs).  First exp fires ~10us (vs 21us in v3): stage A only
    computes q0 + k(blocks 0..7), both from the xa half.
  - scores stay fp32r (cpr=1 at 512 moving), exp output bf16: precision
    identical to v3.  attnv transposed trick (stationary=exp, moving=
    v[128,65] with ones column) unchanged.
  - fillers levelled across the stream via cost-annotated generators:
    h0: q1 + k0B + v; h1: k1 + v; h2: attnv(h0,h1) + norms + v + tp0;
    h3: attnv(h2) + attnv(h3) inline + proj0(pair0).  Tail: last attnv
    in halves off a split final exp, per-qb norm->transpose->proj1 chain.
"""

import os
import numpy as np
import ml_dtypes

import concourse.bacc as bacc
import concourse.mybir as mybir
import concourse.tile as tile
from concourse.bass_utils import run_bass_kernel_spmd
from concourse.masks import make_identity

F32 = mybir.dt.float32
F32R = mybir.dt.float32r
BF16 = mybir.dt.bfloat16
F8 = mybir.dt.float8e4

C = 1024          # model dim
N = 2048          # kv tokens
NQ = 1024         # query tokens
HPC = 4           # heads per core
D = 64            # head dim
DH = HPC * D      # per-core slice of C (256)
P = 128
WS = 64.0         # fp8 weight pre-scale (keeps w out of fp8 subnormals)
XS = 16.0         # fp8 x pre-scale (keeps x RESIDUALS out of subnormals:
                  # hw flushes fp8 subnormal inputs to zero)
SCALE_EXP = (D ** -0.5) / (WS * WS * XS * XS)
DR = mybir.MatmulPerfMode.DoubleRow

_CACHE: dict = {}
_BF = ml_dtypes.bfloat16
_F8 = ml_dtypes.float8_e4m3


def _build():
    nc = bacc.Bacc("TRN2", target_bir_lowering=False, debug=False, num_devices=8)

    # all inputs partition-major: [p, a, ...] with a = C-chunk index
    xah = nc.dram_tensor("xah", [P, 8, NQ], F8, kind="ExternalInput").ap()
    xal = nc.dram_tensor("xal", [P, 8, NQ], F8, kind="ExternalInput").ap()
    xbh = nc.dram_tensor("xbh", [P, 8, NQ], F8, kind="ExternalInput").ap()
    xbl = nc.dram_tensor("xbl", [P, 8, NQ], F8, kind="ExternalInput").ap()
    w0h = nc.dram_tensor("w0h", [P, 8, 2 * P], F8, kind="ExternalInput").ap()
    w0l = nc.dram_tensor("w0l", [P, 8, 2 * P], F8, kind="ExternalInput").ap()
    w1h = nc.dram_tensor("w1h", [P, 8, 2 * P], F8, kind="ExternalInput").ap()
    w1l = nc.dram_tensor("w1l", [P, 8, 2 * P], F8, kind="ExternalInput").ap()
    wvh = nc.dram_tensor("wvh", [P, 8, DH], F8, kind="ExternalInput").ap()
    wvl = nc.dram_tensor("wvl", [P, 8, DH], F8, kind="ExternalInput").ap()
    wp = nc.dram_tensor("wp", [P, 2, C], BF16, kind="ExternalInput").ap()
    outA = nc.dram_tensor("outA", [NQ, C], BF16, kind="ExternalOutput").ap()
    outB = nc.dram_tensor("outB", [NQ, C], BF16, kind="ExternalOutput").ap()

    with tile.TileContext(nc) as tc, \
            nc.allow_low_precision(reason="bf16/fp8-hi-lo pipeline within 2e-2"):
        _emit(tc, xah, xal, xbh, xbl, w0h, w0l, w1h, w1l, wvh, wvl, wp,
              outA, outB)

    nc.compile()
    return nc


def _emit(tc, xah, xal, xbh, xbl, w0h, w0l, w1h, w1l, wvh, wvl, wp,
          outA, outB):
    nc = tc.nc
    mm = nc.tensor.matmul
    Exp = mybir.ActivationFunctionType.Exp
    Copy = mybir.ActivationFunctionType.Copy

    from contextlib import ExitStack
    from itertools import chain

    with ExitStack() as ctx:
        singles = ctx.enter_context(tc.tile_pool(name="singles", bufs=1))
        ets_pool = ctx.enter_context(tc.tile_pool(name="ets", bufs=32))
        finp = ctx.enter_context(tc.tile_pool(name="finp", bufs=8))
        ps_sc = ctx.enter_context(tc.tile_pool(name="ps_sc", bufs=2, space="PSUM"))
        ps_av = ctx.enter_context(tc.tile_pool(name="ps_av", bufs=2, space="PSUM"))
        ps_sq = ctx.enter_context(tc.tile_pool(name="ps_sq", bufs=2, space="PSUM"))

        # ---------------- SBUF inputs ----------------
        xa_h = singles.tile([P, 8, NQ], F8, name="xa_h", tag="xa_h")
        xa_l = singles.tile([P, 8, NQ], F8, name="xa_l", tag="xa_l")
        xb_h = singles.tile([P, 8, NQ], F8, name="xb_h", tag="xb_h")
        xb_l = singles.tile([P, 8, NQ], F8, name="xb_l", tag="xb_l")
        w0h_sb = singles.tile([P, 8, 2 * P], F8, name="w0h_sb", tag="w0h")
        w0l_sb = singles.tile([P, 8, 2 * P], F8, name="w0l_sb", tag="w0l")
        w1h_sb = singles.tile([P, 8, 2 * P], F8, name="w1h_sb", tag="w1h")
        w1l_sb = singles.tile([P, 8, 2 * P], F8, name="w1l_sb", tag="w1l")
        wvh_sb = singles.tile([P, 8, DH], F8, name="wvh_sb", tag="wvh")
        wvl_sb = singles.tile([P, 8, DH], F8, name="wvl_sb", tag="wvl")
        wp_sb = singles.tile([P, 2, C], BF16, name="wp_sb", tag="wp")

        # ---------------- PE warm-up ----------------
        # The cost model's p-state ramp runs from the start of the current
        # continuous PE-busy stretch; full clock needs 3us of ramp.  A chain
        # of dummy matmuls starting at ~0.2us keeps the ramp anchored so the
        # DMA-gated stage-A matmuls (from ~5us) run at 2.4GHz.
        warm_t = singles.tile([P, P], BF16, name="warm_t", tag="warm")
        nc.vector.memset(warm_t, 0.0)
        ps_warm = ps_av.tile([P, P], F32, name="ps_warm", tag="av")

        def warm(n):
            for _ in range(n):
                mm(ps_warm, warm_t, warm_t, start=True, stop=True,
                   skip_group_check=True)

        warm(44)

        # ---------------- input DMAs (one ordered SP/HWDGE stream) --------
        # chunk-pair granularity: each DMA is 128 x 2KB descriptors.
        nc.sync.dma_start(out=w0h_sb, in_=w0h)
        nc.sync.dma_start(out=w0l_sb, in_=w0l)
        for p in range(4):
            s = slice(2 * p, 2 * p + 2)
            nc.sync.dma_start(out=xa_h[:, s, :], in_=xah[:, s, :])
            nc.sync.dma_start(out=xa_l[:, s, :], in_=xal[:, s, :])
        for p in range(4):
            s = slice(2 * p, 2 * p + 2)
            nc.sync.dma_start(out=xb_h[:, s, :], in_=xbh[:, s, :])
            nc.sync.dma_start(out=xb_l[:, s, :], in_=xbl[:, s, :])
        nc.sync.dma_start(out=wvh_sb, in_=wvh)
        nc.sync.dma_start(out=wvl_sb, in_=wvl)
        nc.sync.dma_start(out=w1h_sb, in_=w1h)
        nc.sync.dma_start(out=w1l_sb, in_=w1l)
        nc.sync.dma_start(out=wp_sb, in_=wp)

        # ---------------- small consts ----------------
        identity = singles.tile([P, P], BF16, name="identity", tag="ident")
        make_identity(nc, identity)

        # Pre-trigger the exp table load while DMAs stream.
        dmt = singles.tile([1, 1], BF16, name="dmt", tag="dmt")
        nc.scalar.activation(out=dmt, in_=identity[0:1, 0:1], func=Exp, scale=1.0)

        # ---------------- persistent SBUF ----------------
        qt = [singles.tile([P, NQ], F32R, name=f"qt{p}", tag=f"qt{p}")
              for p in range(2)]
        kt = [singles.tile([P, N], F32R, name=f"kt{p}", tag=f"kt{p}")
              for p in range(2)]
        v_sb = singles.tile([P, 16, HPC, D + 1], BF16, name="v_sb", tag="v_sb")
        nc.gpsimd.memset(v_sb[:, :, :, D:D + 1], 1.0)

        attn_pack = [singles.tile([P, 8, P], BF16, name=f"apk{p}", tag=f"apk{p}")
                     for p in range(2)]
        attn_T = [singles.tile([P, 8, P], BF16, name=f"atT{p}", tag=f"atT{p}")
                  for p in range(2)]
        rcp = singles.tile([P, HPC, 8], F32, name="rcp", tag="rcp")

        # ---------------- 3-pass fp8 hi/lo matmul helper ------------------
        # out accumulates xh@wh + xh@wl + xl@wh over ci-pairs.
        # Emits (mm_fn, cost_ns) items; caller drives issue order.
        def mm3(ps_out, wh, wl, xh, xl, wcol, xcol, first, last):
            # wcol: slice of weight free dim (stationary cols -> out rows)
            # xcol: slice of x token dim; splits moving into <=512 chunks
            t0, t1 = xcol.start, xcol.stop
            splits = [(t, min(t + 256, t1)) for t in range(t0, t1, 256)]
            n = 0
            total = 4 * 3 * len(splits)
            for p in range(4):
                s = slice(2 * p, 2 * p + 2)
                for wsrc, xsrc in ((wh, xh), (wl, xh), (wh, xl)):
                    for (a, b) in splits:
                        st = dict(start=(first and n == 0),
                                  stop=(last and n == total - 1),
                                  skip_group_check=True)
                        mm(ps_out[:, a - t0:b - t0],
                           wsrc[:, s, wcol], xsrc[:, s, a:b],
                           perf_mode=DR, **st)
                        n += 1

        # ---------------- stage A: q0 + k blocks 0..7 ---------------------
        # per ci-pair: 24 DoubleRow mms (3072 cycles = 1.28us) vs 1.46us DMA
        # cadence -> DMA-paced.  hi passes issue before the lo pass so PE
        # starts as soon as the hi chunk lands.
        ps_q0 = ps_sc.tile([P, NQ], F32, name="ps_q0", tag="sc")
        ps_k0 = ps_sc.tile([P, NQ], F32, name="ps_k0", tag="sc")
        for p in range(4):
            s = slice(2 * p, 2 * p + 2)
            first, last = (p == 0), (p == 3)
            n = 0
            for wsrc, xsrc in ((w0h_sb, xa_h), (w0l_sb, xa_h), (w0h_sb, xa_l)):
                for t4 in range(4):
                    ts = slice(t4 * 256, (t4 + 1) * 256)
                    stq = dict(start=(first and n == 0),
                               stop=(last and n == 11), skip_group_check=True)
                    mm(ps_q0[:, ts], wsrc[:, s, 0:P], xsrc[:, s, ts],
                       perf_mode=DR, **stq)
                    mm(ps_k0[:, ts], wsrc[:, s, P:2 * P], xsrc[:, s, ts],
                       perf_mode=DR, **stq)
                    n += 1
            if not last:
                warm(3)  # bridge the DMA-cadence gap, keep p-state pinned
        # evacs: ACT q-half0 (sj0 mm1 gate); DVE k-block0, q-half1 (mm2
        # gate), then the k remainder in halves (sj1 gates on the first).
        # NOTE the scheduler serializes readers of the same psum tile, so
        # ACT and DVE never touch the same tile concurrently here.
        nc.scalar.copy(qt[0][:, 0:512], ps_q0[:, 0:512])
        nc.vector.tensor_copy(kt[0][:, 0:P], ps_k0[:, 0:P])
        nc.vector.tensor_copy(qt[0][:, 512:1024], ps_q0[:, 512:1024])
        nc.vector.tensor_copy(kt[0][:, P:512], ps_k0[:, P:512])
        nc.vector.tensor_copy(kt[0][:, 512:1024], ps_k0[:, 512:1024])

        # ---------------- fillers (cost-annotated generators) -------------
        def g_q1():
            # q pair1 [128, 1024]: 2 half-psums, 24 DoubleRow mms each
            for g in range(2):
                ps = ps_sq.tile([P, 512], F32, name=f"ps_q1{g}", tag="sq")
                n = 0
                for p in range(4):
                    s = slice(2 * p, 2 * p + 2)
                    for wsrc, xsrc in ((w1h_sb, xa_h), (w1l_sb, xa_h),
                                       (w1h_sb, xa_l)):
                        for t2 in range(2):
                            ts = slice(g * 512 + t2 * 256,
                                       g * 512 + (t2 + 1) * 256)
                            mm(ps[:, t2 * 256:(t2 + 1) * 256],
                               wsrc[:, s, 0:P], xsrc[:, s, ts],
                               perf_mode=DR, start=(n == 0), stop=(n == 23),
                               skip_group_check=True)
                            n += 1
                            yield 53
                nc.vector.tensor_copy(qt[1][:, g * 512:(g + 1) * 512], ps)
                yield 0

        def g_k(pair, half):
            # k blocks for (pair, token-half): kt[pair][:, half*1024:...]
            wsb_h = w0h_sb if pair == 0 else w1h_sb
            wsb_l = w0l_sb if pair == 0 else w1l_sb
            xh = xa_h if half == 0 else xb_h
            xl = xa_l if half == 0 else xb_l
            for g in range(2):
                ps = ps_sq.tile([P, 512], F32, name=f"ps_k{pair}{half}{g}",
                                tag="sq")
                n = 0
                for p in range(4):
                    s = slice(2 * p, 2 * p + 2)
                    for wsrc, xsrc in ((wsb_h, xh), (wsb_l, xh), (wsb_h, xl)):
                        for t2 in range(2):
                            ts = slice(g * 512 + t2 * 256,
                                       g * 512 + (t2 + 1) * 256)
                            mm(ps[:, t2 * 256:(t2 + 1) * 256],
                               wsrc[:, s, P:2 * P], xsrc[:, s, ts],
                               perf_mode=DR, start=(n == 0), stop=(n == 23),
                               skip_group_check=True)
                            n += 1
                            yield 53
                nc.vector.tensor_copy(
                    kt[pair][:, half * NQ + g * 512:half * NQ + (g + 1) * 512],
                    ps)
                yield 0

        def g_v(t):
            # kv token blocks 2t, 2t+1 -> v_sb (x slice stationary, wv moving)
            ps = ps_sq.tile([P, 2, DH], F32, name=f"ps_v{t}", tag="sq")
            for jj in range(2):
                jb = 2 * t + jj
                xh = xa_h if jb < 8 else xb_h
                xl = xa_l if jb < 8 else xb_l
                ts = slice((jb % 8) * P, (jb % 8) * P + P)
                n = 0
                for p in range(4):
                    s = slice(2 * p, 2 * p + 2)
                    for wsrc, xsrc in ((wvh_sb, xh), (wvl_sb, xh),
                                       (wvh_sb, xl)):
                        mm(ps[:, jj, :], xsrc[:, s, ts], wsrc[:, s, :],
                           perf_mode=DR,
                           start=(jj == 0 and n == 0),
                           stop=(jj == 1 and n == 11),
                           skip_group_check=True)
                        n += 1
                        yield 53
            nc.vector.tensor_copy(
                v_sb[:, 2 * t:2 * t + 2, :, 0:D],
                ps.rearrange("p j (h d) -> p j h d", h=HPC))
            yield 0

        def g_proj0(m):
            # pair0 projection of q-block m -> outA (2 half-column units)
            for nh in range(2):
                ps = ps_sq.tile([P, 512], F32, name=f"ps_pj0_{m}_{nh}", tag="sq")
                mm(ps, attn_T[0][:, m, :],
                   wp_sb[:, 0, nh * 512:(nh + 1) * 512],
                   start=True, stop=True, skip_group_check=True)
                yield 213
                fin = finp.tile([P, 512], BF16, name=f"fin0_{m}_{nh}", tag="fin")
                nc.vector.tensor_copy(fin, ps)
                nc.sync.dma_start(
                    out=outA[m * P:(m + 1) * P, nh * 512:(nh + 1) * 512],
                    in_=fin)
                yield 0

        # ---------------- attention pieces ----------------
        av_tiles = {}

        def alloc_av(h):
            av_tiles[h] = [ps_av.tile([P, 4, D + 1], F32, name=f"av{h}_{s}",
                                      tag="av") for s in range(2)]

        ets = {}

        def scores_j(h, j):
            pair, po = h // 2, D * (h % 2)
            ps = ps_sc.tile([P, NQ], F32, name=f"ps_s{h}_{j}", tag="sc")
            lw = kt[pair][po:po + D, j * P:(j + 1) * P]
            for nh in range(2):
                mm(ps[:, nh * 512:(nh + 1) * 512], lw,
                   qt[pair][po:po + D, nh * 512:(nh + 1) * 512],
                   start=True, stop=True, skip_group_check=True)
            et = ets_pool.tile([P, NQ], BF16, name=f"et{h}_{j}", tag="ets")
            if (h, j) in ((0, 0), (3, 15)):
                # split exps at the stream edges: (0,0) starts on the first
                # q-half evac; (3,15) releases the tail's first attnv half
                # half an exp earlier
                nc.scalar.activation(out=et[:, 0:512], in_=ps[:, 0:512],
                                     func=Exp, scale=SCALE_EXP)
                nc.scalar.activation(out=et[:, 512:1024], in_=ps[:, 512:1024],
                                     func=Exp, scale=SCALE_EXP)
            else:
                nc.scalar.activation(out=et, in_=ps, func=Exp, scale=SCALE_EXP)
            ets[(h, j)] = et

        def attnv_j(h, j):
            et = ets[(h, j)]
            for qb in range(8):
                av = av_tiles[h][qb // 4]
                mm(av[:, qb % 4, :],
                   et[:, qb * P:(qb + 1) * P],
                   v_sb[:, j, h, :],
                   start=(j == 0 and qb % 4 == 0),
                   stop=(j == 15 and qb % 4 == 3),
                   skip_group_check=True)

        def norm_half(h, part, tail):
            pair, half = h // 2, h % 2
            av = av_tiles[h][part]
            nc.vector.reciprocal(rcp[:, h, part * 4:(part + 1) * 4], av[:, :, D])
            for i in range(4):
                qb = part * 4 + i
                dst = attn_pack[pair][:, qb, half * D:(half + 1) * D]
                if tail and i % 2 == 1:
                    nc.scalar.activation(out=dst, in_=av[:, i, 0:D], func=Copy,
                                         scale=rcp[:, h, qb:qb + 1])
                else:
                    nc.vector.tensor_scalar_mul(dst, av[:, i, 0:D],
                                                rcp[:, h, qb:qb + 1])

        def pull(gen, budget):
            acc = 0
            while acc < budget:
                c = next(gen, None)
                if c is None:
                    return False
                acc += c
            return True

        def tp0(qb):
            tp = ps_av.tile([P, P], BF16, name=f"tp0_{qb}", tag="av")
            nc.tensor.transpose(tp, attn_pack[0][:, qb, :], identity)
            nc.vector.tensor_copy(attn_T[0][:, qb, :], tp)

        def g_pause(n):
            for _ in range(n):
                yield 640

        # ---------------- head loops (ACT exp stream is the pacer) --------
        # budgets: ~640ns/iter of PE filler vs 1068ns exp cadence.
        # Filler chains ordered by DMA arrival: k0B (xb), v (wv), q1/k1 (w1).
        f = chain(g_pause(2), g_k(0, 1), g_v(0), g_v(1), g_q1())
        for j in range(16):
            scores_j(0, j)
            pull(f, 610)
        for _ in f:
            pass

        # h1: v blocks with k1 first half slotted mid-head (kt[1] blocks
        # 0..7 must land before h2 j0)
        f = chain(g_v(2), g_v(3), g_k(1, 0), g_v(4), g_v(5), g_v(6),
                  g_v(7))
        for j in range(16):
            scores_j(1, j)
            pull(f, 640)
        for _ in f:
            pass

        # h2: k1 second half pulled j0-6 (kt[1] blocks 8..15 needed by h2
        # j8); attnv(h0) spread 2/iter, norm(h0)@j6, attnv(h1) j7..j14,
        # norm(h1)@j15.
        A1 = [(0, 2), (2, 4), (4, 7), (7, 9), (9, 11), (11, 13), (13, 16)]
        alloc_av(0)
        fv = chain(g_k(1, 1))

        for j in range(16):
            scores_j(2, j)
            if j < 8:
                attnv_j(0, 2 * j)
                attnv_j(0, 2 * j + 1)
            elif j == 8:
                norm_half(0, 0, False)
                norm_half(0, 1, False)
                alloc_av(1)
            else:
                for jj in range(*A1[j - 9]):
                    attnv_j(1, jj)
            if j == 15:
                norm_half(1, 0, False)
                norm_half(1, 1, False)
            if j < 7:
                pull(fv, 400)

        # h3: tp0, attnv(h2) ~2/iter, norm(h2)@8-9, attnv(h3) inline,
        # proj0 spread j2..15
        alloc_av(2)
        fp = chain(*(g_proj0(m) for m in range(8)))
        for j in range(16):
            scores_j(3, j)
            if j < 4:
                tp0(2 * j)
                tp0(2 * j + 1)
            if j < 8:
                attnv_j(2, 2 * j)
                attnv_j(2, 2 * j + 1)
            else:
                if j == 8:
                    norm_half(2, 0, False)
                elif j == 9:
                    norm_half(2, 1, False)
                    alloc_av(3)
                if j >= 9:
                    for jj in range((j - 9) * 15 // 7, (j - 8) * 15 // 7):
                        attnv_j(3, jj)
            if 4 <= j <= 7:
                pull(fp, 360)
            elif j >= 8:
                pull(fp, 500)
        for _ in fp:
            pass

        # ---------------- tail ----------------
        et15 = ets[(3, 15)]
        for qb in range(4):
            mm(av_tiles[3][qb // 4][:, qb % 4, :],
               et15[:, qb * P:(qb + 1) * P], v_sb[:, 15, 3, :],
               start=False, stop=(qb == 3), skip_group_check=True)

        av3 = av_tiles[3]
        nc.vector.reciprocal(rcp[:, 3, 0:4], av3[0][:, :, D])

        def mul3(qb):
            dst = attn_pack[1][:, qb, D:2 * D]
            src_ = av3[qb // 4][:, qb % 4, 0:D]
            nc.vector.tensor_scalar_mul(dst, src_, rcp[:, 3, qb:qb + 1])

        def tp1(qb):
            tp = ps_sq.tile([P, P], BF16, name=f"tp{qb}", tag="sq")
            nc.tensor.transpose(tp, attn_pack[1][:, qb, :], identity)
            if qb % 2 == 0:
                nc.vector.tensor_copy(attn_T[1][:, qb, :], tp)
            else:
                nc.scalar.copy(attn_T[1][:, qb, :], tp)

        def proj1(m):
            fin = finp.tile([P, C], BF16, name=f"fin1_{m}", tag="fin")
            ps = ps_sc.tile([P, NQ], F32, name=f"pj1_{m}", tag="sc")
            for nh in range(2):
                mm(ps[:, nh * 512:(nh + 1) * 512],
                   attn_T[1][:, m, :],
                   wp_sb[:, 1, nh * 512:(nh + 1) * 512],
                   start=True, stop=True, skip_group_check=True)
            if m % 2 == 0:
                nc.scalar.copy(fin, ps)
            else:
                nc.vector.tensor_copy(fin, ps)
            nc.sync.dma_start(out=outB[m * P:(m + 1) * P, :], in_=fin)

        mul3(0)
        tp1(0)
        mul3(1)
        tp1(1)
        for qb in range(4, 8):
            mm(av_tiles[3][qb // 4][:, qb % 4, :],
               et15[:, qb * P:(qb + 1) * P], v_sb[:, 15, 3, :],
               start=False, stop=(qb == 7), skip_group_check=True)
        nc.vector.reciprocal(rcp[:, 3, 4:8], av3[1][:, :, D])
        for qb in range(2, 8):
            mul3(qb)
            tp1(qb)
            proj1(qb - 2)
        proj1(6)
        proj1(7)


def _get_nc():
    if "nc" not in _CACHE:
        _CACHE["nc"] = _build()
    return _CACHE["nc"]


def _split8(a):
    """[1024, X] f32 -> (hi, lo) fp8 in partition-major [128, 8, X]."""
    h = a.astype(_F8)
    lo = (a - h.astype(np.float32)).astype(_F8)

    def sw(t):
        return np.ascontiguousarray(
            t.reshape(8, P, -1).transpose(1, 0, 2))
    return sw(h), sw(lo)


def kernel(x, wq, wk, wv, w_proj, b_proj):
    x = np.asarray(x, dtype=np.float32)
    wq = np.asarray(wq, dtype=np.float32)
    wk = np.asarray(wk, dtype=np.float32)
    wv = np.asarray(wv, dtype=np.float32)
    w_proj = np.asarray(w_proj, dtype=np.float32)
    b_proj = np.asarray(b_proj, dtype=np.float32)

    nc = _get_nc()
    in_maps = []
    for core in range(8):
        b, g = divmod(core, 4)
        p0 = slice(g * DH, g * DH + P)          # pair0 rows (heads 4g, 4g+1)
        p1 = slice(g * DH + P, g * DH + 2 * P)  # pair1 rows
        sl = slice(g * DH, (g + 1) * DH)

        xT = np.ascontiguousarray(x[b].T) * XS  # [1024 C, 2048 tok]
        xah_, xal_ = _split8(xT[:, :NQ])
        xbh_, xbl_ = _split8(xT[:, NQ:])
        w0 = np.hstack([wq[p0, :].T, wk[p0, :].T]) * WS
        w1 = np.hstack([wq[p1, :].T, wk[p1, :].T]) * WS
        w0h_, w0l_ = _split8(w0)
        w1h_, w1l_ = _split8(w1)
        wvh_, wvl_ = _split8(wv[sl, :].T * WS)
        wpT = (w_proj[:, sl].T / (WS * XS)).astype(_BF)  # [256, 1024]
        wp_pm = np.ascontiguousarray(
            wpT.reshape(2, P, C).transpose(1, 0, 2))

        in_maps.append({
            "xah": xah_, "xal": xal_, "xbh": xbh_, "xbl": xbl_,
            "w0h": w0h_, "w0l": w0l_, "w1h": w1h_, "w1l": w1l_,
            "wvh": wvh_, "wvl": wvl_, "wp": wp_pm,
        })

    res = run_bass_kernel_spmd(nc, in_maps, core_ids=list(range(8)),
                               trace=bool(int(os.environ.get("KERNEL_TRACE", "0"))))
    _CACHE["last_results"] = res
    acc = [np.zeros((NQ, C), np.float32) for _ in range(2)]
    for core in range(8):
        b = core // 4
        acc[b] += res.results[core]["outA"].astype(np.float32)
        acc[b] += res.results[core]["outB"].astype(np.float32)
    full = np.stack(acc)
    full += b_proj[None, None, :]
    return full.astype(np.float32)
